# revision 1
# baseline (speedup 1.0000x reference)
"""GraphSAGE 2-layer kernel for Trainium2, 8 NeuronCores, data-parallel over nodes.

Strategy:
- Nodes padded to 50176 = 8 cores * 49 tiles * 128; each core owns 6272 rows.
- One compiled Bass program = one SAGE layer for one core's slice:
    out[n, :] = relu(x[n] @ W_top + (mean_k x[nbr[n,k]]) @ W_bot + b)
  run twice (layer 1 on x, layer 2 on h1) with a host-side gather between.
- Per 128-node tile:
    * one indirect DMA gathers all 16 neighbor rows (2048 descriptors x 512B)
    * neighbor sum split between DVE (tensor_add) and PE (transpose-accumulate
      into PSUM via matmul with identity), mean folded into pre-scaled W_bot
    * self term uses a host-transposed slice of x so lhsT needs no on-chip
      transpose; two matmuls accumulate x@W_top + s@W_bot_scaled in PSUM
    * bias (host-broadcast to [128,128]) added on DVE, relu on ACT engine
"""

import sys

sys.path.insert(0, "/opt/trn_rl_repo")

from contextlib import ExitStack

import numpy as np

import concourse.bass as bass
import concourse.tile as tile
from concourse import mybir
from concourse.bass_utils import run_bass_kernel_spmd
from concourse.masks import make_identity

P = 128
DEG = 16
C = 8
N_DVE = 10  # neighbor blocks summed on DVE; rest transposed+summed on PE

_NC_CACHE = {}


def _split_wide_waits(nc, max_waits=1):
    """walrus codegen here allows a single sync-wait per instruction; move
    extra waits onto preceding nops on the same engine queue."""
    for fn in nc.m.functions:
        for bb in fn.blocks:
            out = []
            for inst in bb.instructions:
                si = inst.sync_info
                ow = list(si.on_wait) if si and si.on_wait else []
                limit = 0 if isinstance(inst, mybir.InstDrain) else max_waits
                if len(ow) > limit:
                    extra = ow if limit == 0 else ow[:-limit]
                    keep = [] if limit == 0 else ow[-limit:]
                    for k in range(0, len(extra), max_waits):
                        out.append(
                            mybir.InstNoOp(
                                name=f"{inst.name}-waitsplit{k}",
                                opcode="Nop",
                                engine=inst.engine,
                                debug=inst.debug,
                                ins=[],
                                outs=[],
                                sync_info=mybir.SyncInfo(
                                    on_wait=extra[k : k + max_waits], on_update=[]
                                ),
                                text_hint="waitsplit",
                                bass_nofuse=True,
                            )
                        )
                    si.on_wait = keep
                out.append(inst)
            bb.instructions[:] = out


def build_layer_nc(n_tiles, n_full):
    key = (n_tiles, n_full)
    if key in _NC_CACHE:
        return _NC_CACHE[key]
    f32 = mybir.dt.float32
    i32 = mybir.dt.int32
    nc = bass.Bass("TRN2", target_bir_lowering=False, debug=False, num_devices=C)
    xfull = nc.dram_tensor("xfull", [n_full, P], f32, kind="ExternalInput").ap()
    xt = nc.dram_tensor("xt", [P, n_tiles * P], f32, kind="ExternalInput").ap()
    nbr = nc.dram_tensor("nbr", [P, n_tiles * DEG], i32, kind="ExternalInput").ap()
    wt = nc.dram_tensor("wt", [P, P], f32, kind="ExternalInput").ap()
    wb = nc.dram_tensor("wb", [P, P], f32, kind="ExternalInput").ap()
    bbc = nc.dram_tensor("bbc", [P, P], f32, kind="ExternalInput").ap()
    out = nc.dram_tensor("out", [n_tiles * P, P], f32, kind="ExternalOutput").ap()

    with tile.TileContext(nc) as tc:
        with ExitStack() as ctx:
            consts = ctx.enter_context(tc.tile_pool(name="consts", bufs=1))
            gpool = ctx.enter_context(tc.tile_pool(name="gath", bufs=6))
            spool = ctx.enter_context(tc.tile_pool(name="sums", bufs=8))
            hpool = ctx.enter_context(tc.tile_pool(name="hh", bufs=4))
            xtp = ctx.enter_context(tc.tile_pool(name="xtp", bufs=4))
            nbrp = ctx.enter_context(tc.tile_pool(name="nbrp", bufs=8))
            ps_t = ctx.enter_context(tc.tile_pool(name="ps_t", bufs=3, space="PSUM"))
            ps_h = ctx.enter_context(tc.tile_pool(name="ps_h", bufs=3, space="PSUM"))

            ident = consts.tile([P, P], f32)
            make_identity(nc, ident[:])
            wt_sb = consts.tile([P, P], f32)
            nc.sync.dma_start(wt_sb[:], wt[:, :])
            wb_sb = consts.tile([P, P], f32)
            nc.sync.dma_start(wb_sb[:], wb[:, :])
            bb_sb = consts.tile([P, P], f32)
            nc.sync.dma_start(bb_sb[:], bbc[:, :])
            nbr_sb = consts.tile([P, n_tiles * DEG], i32)
            nc.sync.dma_start(nbr_sb[:], nbr[:, :])

            for t in range(n_tiles):
                g = gpool.tile([P, DEG * P], f32)
                for k in range(DEG):
                    nc.gpsimd.indirect_dma_start(
                        out=g[:, k * P : (k + 1) * P],
                        out_offset=None,
                        in_=xfull[:, :],
                        in_offset=bass.IndirectOffsetOnAxis(
                            ap=nbr_sb[:, t * DEG + k : t * DEG + k + 1], axis=0
                        ),
                    )
                acc = spool.tile([P, P], f32)
                nc.vector.tensor_add(acc[:], g[:, 0:P], g[:, P : 2 * P])
                for k in range(2, N_DVE):
                    nc.vector.tensor_add(acc[:], acc[:], g[:, k * P : (k + 1) * P])
                pst = ps_t.tile([P, P], f32)
                nc.tensor.matmul(
                    out=pst[:], lhsT=acc[:], rhs=ident[:], start=True, stop=False
                )
                for k in range(N_DVE, DEG):
                    nc.tensor.matmul(
                        out=pst[:],
                        lhsT=g[:, k * P : (k + 1) * P],
                        rhs=ident[:],
                        start=False,
                        stop=(k == DEG - 1),
                    )
                sT = spool.tile([P, P], f32)
                nc.scalar.copy(sT[:], pst[:])
                xt_t = xtp.tile([P, P], f32)
                nc.sync.dma_start(xt_t[:], xt[:, t * P : (t + 1) * P])
                psh = ps_h.tile([P, P], f32)
                nc.tensor.matmul(
                    out=psh[:], lhsT=xt_t[:], rhs=wt_sb[:], start=True, stop=False
                )
                nc.tensor.matmul(
                    out=psh[:], lhsT=sT[:], rhs=wb_sb[:], start=False, stop=True
                )
                hb = hpool.tile([P, P], f32)
                nc.vector.tensor_add(hb[:], psh[:], bb_sb[:])
                h = hpool.tile([P, P], f32)
                nc.scalar.activation(h[:], hb[:], mybir.ActivationFunctionType.Relu)
                nc.sync.dma_start(out[t * P : (t + 1) * P, :], h[:])

    _split_wide_waits(nc)
    _NC_CACHE[key] = nc
    return nc


def _run_layer(nc, xp, nbrp_arr, W, b, n_tiles, npc, trace=False):
    """xp: [n_full, P] f32 table; nbrp_arr: [n_full, DEG] i32. Returns [n_full, P]."""
    wt = np.ascontiguousarray(W[:P, :], dtype=np.float32)
    wb = np.ascontiguousarray(W[P:, :], dtype=np.float32) / np.float32(DEG)
    bbc = np.tile(np.asarray(b, dtype=np.float32).reshape(1, P), (P, 1))
    in_maps = []
    for c in range(C):
        sl = slice(c * npc, (c + 1) * npc)
        in_maps.append(
            {
                "xfull": xp,
                "xt": np.ascontiguousarray(xp[sl].T),
                "nbr": np.ascontiguousarray(
                    nbrp_arr[sl]
                    .reshape(-1, 128, DEG)
                    .transpose(1, 0, 2)
                    .reshape(128, -1)
                ),
                "wt": wt,
                "wb": wb,
                "bbc": bbc,
            }
        )
    res = run_bass_kernel_spmd(nc, in_maps, core_ids=list(range(C)), trace=trace)
    h = np.concatenate([res.results[c]["out"] for c in range(C)], axis=0)
    return h, res


LAST_RUNS = []


def kernel(x, neighbors, W1, b1, W2, b2):
    N, D = x.shape
    assert D == P
    npc = -(-N // (C * P)) * P  # rows per core, padded to 128
    n_full = C * npc
    n_tiles = npc // P

    xp = np.zeros((n_full, P), dtype=np.float32)
    xp[:N] = np.asarray(x, dtype=np.float32)
    nbrp_arr = np.zeros((n_full, DEG), dtype=np.int32)
    nbrp_arr[:N] = np.asarray(neighbors).astype(np.int32)

    nc = build_layer_nc(n_tiles, n_full)
    h1, r1 = _run_layer(nc, xp, nbrp_arr, W1, b1, n_tiles, npc)
    out, r2 = _run_layer(nc, h1, nbrp_arr, W2, b2, n_tiles, npc)
    LAST_RUNS[:] = [r1, r2]
    return out[:N]



# revision 6
# speedup vs baseline: 2.5914x; 2.5914x over previous
"""GraphSAGE 2-layer kernel for Trainium2, 8 NeuronCores, data-parallel over nodes.

Strategy (v2 — dma_gather):
- Nodes padded to 50176 = 8 cores * 49 tiles * 128; each core owns 6272 rows.
- One compiled Bass program = one SAGE layer for one core's slice, computing
    out[:, n] = relu(W_top^T x_n + W_bot_scaled^T (sum_k x[nbr[n,k]]) + b)
  i.e. the OUTPUT IS TRANSPOSED [128 outf, npc nodes]; host stitches/casts
  between the two layer launches (host work is not on the HW critical path).
- Neighbor gather via the mlp-library dma_gather (InstDMAGatherAnt):
  * table packed in pairs: t2[q] = concat(x[2q], x[2q+1]) (512B f16 rows), so
    the int16 gather index q = node//2 stays < 32768; a parity mask selects
    the wanted 256B half on DVE after the gather.
  * two 1024-index gathers per 128-node tile (16 neighbors each); 1024 is the
    max num_idxs that executes reliably per instruction.
- Sum over 16 neighbors: parity select (copy + copy_predicated) then a 4-level
  tree of wide DVE adds; mean folded into pre-scaled W_bot.
- s^T via PE identity-transpose (f32), then two f16 matmuls accumulate
  W_top^T x + W_bot^T s in PSUM; bias+relu fused on ACT (bias is per-partition
  in the transposed orientation).
"""

import sys

sys.path.insert(0, "/opt/trn_rl_repo")

from contextlib import ExitStack

import numpy as np

import concourse.bass as bass
import concourse.tile as tile
from concourse import library_config, mybir
from concourse.bass_utils import run_bass_kernel_spmd
from concourse.library_overlay import lower_extended_insts
from concourse.masks import make_identity

P = 128
DEG = 16
C = 8
GN = 1024  # indices per dma_gather instruction
PADF = 8  # free-dim pad (f16 elems) after each 128-wide chunk

f32 = mybir.dt.float32
f16 = mybir.dt.float16
i16 = mybir.dt.int16
u8 = mybir.dt.uint8

_NC_CACHE = {}


def _split_wide_waits(nc, max_waits=1):
    """walrus codegen here allows a single sync-wait per instruction; move
    extra waits onto preceding nops on the same engine queue."""
    for fn in nc.m.functions:
        for bb in fn.blocks:
            out = []
            for inst in bb.instructions:
                si = inst.sync_info
                ow = list(si.on_wait) if si and si.on_wait else []
                limit = 0 if isinstance(inst, mybir.InstDrain) else max_waits
                if len(ow) > limit:
                    extra = ow if limit == 0 else ow[:-limit]
                    keep = [] if limit == 0 else ow[-limit:]
                    for k in range(0, len(extra), max_waits):
                        out.append(
                            mybir.InstNoOp(
                                name=f"{inst.name}-waitsplit{k}",
                                opcode="Nop",
                                engine=inst.engine,
                                debug=inst.debug,
                                ins=[],
                                outs=[],
                                sync_info=mybir.SyncInfo(
                                    on_wait=extra[k : k + max_waits], on_update=[]
                                ),
                                text_hint="waitsplit",
                                bass_nofuse=True,
                            )
                        )
                    si.on_wait = keep
                out.append(inst)
            bb.instructions[:] = out


def build_layer_nc(n_tiles, n_full):
    key = (n_tiles, n_full)
    if key in _NC_CACHE:
        return _NC_CACHE[key]
    npc = n_tiles * P
    nc = bass.Bass(
        "TRN2",
        target_bir_lowering=False,
        debug=False,
        num_devices=C,
        num_swdge_queues=4,
    )
    t2 = nc.dram_tensor("t2", [n_full // 2, 2 * P], f16, kind="ExternalInput").ap()
    xt = nc.dram_tensor("xt", [P, npc], f16, kind="ExternalInput").ap()
    idx = nc.dram_tensor("idx", [P, n_tiles * P], i16, kind="ExternalInput").ap()
    par = nc.dram_tensor("par", [P, n_tiles * DEG], u8, kind="ExternalInput").ap()
    wt = nc.dram_tensor("wt", [P, P], f16, kind="ExternalInput").ap()
    wb = nc.dram_tensor("wb", [P, P], f16, kind="ExternalInput").ap()
    bia = nc.dram_tensor("bia", [P, 1], f32, kind="ExternalInput").ap()
    out = nc.dram_tensor("out", [P, npc], f32, kind="ExternalOutput").ap()

    with tile.TileContext(nc) as tc:
        with ExitStack() as ctx:
            consts = ctx.enter_context(tc.tile_pool(name="consts", bufs=1))
            gpool = ctx.enter_context(tc.tile_pool(name="gath", bufs=4))
            selp = ctx.enter_context(tc.tile_pool(name="selp", bufs=2))
            t8p = ctx.enter_context(tc.tile_pool(name="t8p", bufs=2))
            t4p = ctx.enter_context(tc.tile_pool(name="t4p", bufs=2))
            t2p = ctx.enter_context(tc.tile_pool(name="t2p", bufs=2))
            t1p = ctx.enter_context(tc.tile_pool(name="t1p", bufs=2))
            stp = ctx.enter_context(tc.tile_pool(name="stp", bufs=2))
            hpool = ctx.enter_context(tc.tile_pool(name="hh", bufs=3))
            ps_t = ctx.enter_context(tc.tile_pool(name="ps_t", bufs=2, space="PSUM"))
            ps_h = ctx.enter_context(tc.tile_pool(name="ps_h", bufs=3, space="PSUM"))

            nc.gpsimd.load_library(library_config.mlp)
            gn_reg = nc.gpsimd.to_reg(GN)

            ident = consts.tile([P, P], f32)
            make_identity(nc, ident[:])
            wt_sb = consts.tile([P, P], f16)
            nc.sync.dma_start(wt_sb[:], wt[:, :])
            wb_sb = consts.tile([P, P], f16)
            nc.sync.dma_start(wb_sb[:], wb[:, :])
            b_sb = consts.tile([P, 1], f32)
            nc.sync.dma_start(b_sb[:], bia[:, :])
            xt_sb = consts.tile([P, npc], f16)
            nc.sync.dma_start(xt_sb[:], xt[:, :])
            idx_sb = consts.tile([P, n_tiles * P], i16)
            nc.sync.dma_start(idx_sb[:], idx[:, :])
            par_sb = consts.tile([P, n_tiles * DEG], u8)
            nc.sync.dma_start(par_sb[:], par[:, :])

            for t in range(n_tiles):
                g = gpool.tile([P, DEG, 2 * P], f16)
                for h in range(2):
                    nc.gpsimd.dma_gather(
                        out_ap=g[:, 8 * h : 8 * h + 8, :],
                        in_ap=t2[:, :],
                        idxs_ap=idx_sb[:, t * P + 64 * h : t * P + 64 * (h + 1)],
                        num_idxs=GN,
                        num_idxs_reg=gn_reg,
                        elem_size=2 * P,
                        queue_num=(2 * t + h) % 4,
                    )
                # PADF keeps free-dim chunk strides non-contiguous so every
                # DVE operand lowers with the same rank (interp np views).
                sel = selp.tile([P, DEG, P + PADF], f16)
                nc.vector.tensor_copy(sel[:, :, 0:P], g[:, :, 0:P])
                nc.vector.copy_predicated(
                    sel[:, :, 0:P],
                    par_sb[:, t * DEG : (t + 1) * DEG]
                    .unsqueeze(2)
                    .broadcast_to([P, DEG, P]),
                    g[:, :, P : 2 * P],
                )
                s8 = t8p.tile([P, 8, P + PADF], f16)
                nc.vector.tensor_add(
                    s8[:, :, 0:P], sel[:, 0:8, 0:P], sel[:, 8:16, 0:P]
                )
                s4 = t4p.tile([P, 4, P + PADF], f16)
                nc.vector.tensor_add(
                    s4[:, :, 0:P], s8[:, 0:4, 0:P], s8[:, 4:8, 0:P]
                )
                s2 = t2p.tile([P, 2, P + PADF], f16)
                nc.vector.tensor_add(
                    s2[:, :, 0:P], s4[:, 0:2, 0:P], s4[:, 2:4, 0:P]
                )
                s1 = t1p.tile([P, P], f32)
                nc.vector.tensor_add(s1[:], s2[:, 0, 0:P], s2[:, 1, 0:P])

                pst = ps_t.tile([P, P], f32)
                nc.tensor.transpose(pst[:], s1[:], ident[:])
                sT = stp.tile([P, P], f16)
                nc.scalar.copy(sT[:], pst[:])

                psh = ps_h.tile([P, P], f32)
                nc.tensor.matmul(
                    out=psh[:],
                    lhsT=wt_sb[:],
                    rhs=xt_sb[:, t * P : (t + 1) * P],
                    start=True,
                    stop=False,
                )
                nc.tensor.matmul(
                    out=psh[:], lhsT=wb_sb[:], rhs=sT[:], start=False, stop=True
                )
                hb = hpool.tile([P, P], f32)
                nc.scalar.activation(
                    hb[:], psh[:], mybir.ActivationFunctionType.Relu, bias=b_sb[:]
                )
                nc.sync.dma_start(out[:, t * P : (t + 1) * P], hb[:])

    _split_wide_waits(nc)
    lower_extended_insts(nc)
    _NC_CACHE[key] = nc
    return nc


def _make_idx_par(nbr_pad, npc, n_tiles):
    """Per-core gather index + parity layouts.

    nbr_pad: [n_full, DEG] int64/int32 neighbor table (full).
    Returns per-core lists: idx [128, n_tiles*128] i16, par [128, n_tiles*16] u8.
    Slot s (within a 1024-idx gather h of tile t) = k_local*128 + n, with
    k = 8*h + k_local; idx value = nbr//2 placed at
    [s%16 (replicated across the 8 16-partition groups), t*128 + 64*h + s//16].
    """
    idxs, pars = [], []
    for c in range(C):
        nb = nbr_pad[c * npc : (c + 1) * npc].reshape(n_tiles, P, DEG)
        a = (nb // 2).astype(np.int16).transpose(0, 2, 1)  # [t, k, n]
        a = a.reshape(n_tiles, 2, 8, 8, 16)  # [t, h, k_l, n_hi, n_lo]
        b = a.transpose(4, 0, 1, 2, 3).reshape(16, n_tiles * P)
        idxs.append(np.ascontiguousarray(np.tile(b, (8, 1))))
        p = (nb & 1).astype(np.uint8).transpose(1, 0, 2).reshape(P, n_tiles * DEG)
        pars.append(np.ascontiguousarray(p))
    return idxs, pars


def _run_layer(nc, table_f16, xt_slices, idxs, pars, W, b, trace=False):
    wt = np.ascontiguousarray(W[:P, :]).astype(np.float16)
    wb = (np.ascontiguousarray(W[P:, :]) / np.float32(DEG)).astype(np.float16)
    bia = np.asarray(b, dtype=np.float32).reshape(P, 1)
    t2 = table_f16.reshape(table_f16.shape[0] // 2, 2 * P)
    in_maps = []
    for c in range(C):
        in_maps.append(
            {
                "t2": t2,
                "xt": xt_slices[c],
                "idx": idxs[c],
                "par": pars[c],
                "wt": wt,
                "wb": wb,
                "bia": bia,
            }
        )
    res = run_bass_kernel_spmd(nc, in_maps, core_ids=list(range(C)), trace=trace)
    # h^T stitched: [128, n_full] f32
    ht = np.concatenate([res.results[c]["out"] for c in range(C)], axis=1)
    return ht, res


LAST_RUNS = []


def kernel(x, neighbors, W1, b1, W2, b2):
    N, D = x.shape
    assert D == P
    npc = -(-N // (C * P)) * P  # rows per core, padded to 128
    n_full = C * npc
    n_tiles = npc // P

    xp = np.zeros((n_full, P), dtype=np.float32)
    xp[:N] = np.asarray(x, dtype=np.float32)
    nbr_pad = np.zeros((n_full, DEG), dtype=np.int64)
    nbr_pad[:N] = np.asarray(neighbors)

    idxs, pars = _make_idx_par(nbr_pad, npc, n_tiles)
    nc = build_layer_nc(n_tiles, n_full)

    x16 = xp.astype(np.float16)
    xt1 = [
        np.ascontiguousarray(x16[c * npc : (c + 1) * npc].T) for c in range(C)
    ]
    h1t, r1 = _run_layer(nc, x16, xt1, idxs, pars, W1, b1)

    h1_16 = h1t.astype(np.float16)  # [128, n_full]
    table2 = np.ascontiguousarray(h1_16.T)  # [n_full, 128] node-major
    xt2 = [
        np.ascontiguousarray(h1_16[:, c * npc : (c + 1) * npc]) for c in range(C)
    ]
    h2t, r2 = _run_layer(nc, table2, xt2, idxs, pars, W2, b2)

    LAST_RUNS[:] = [r1, r2]
    return np.ascontiguousarray(h2t.T[:N]).astype(np.float32)
